# revision 1
# baseline (speedup 1.0000x reference)
"""Trainium2 Bass kernel for a 2-layer GraphConv block (PyG GraphConv, aggr=add):
    h1  = leaky_relu(segsum(x[src], dst) @ W1_rel.T + b1 + x @ W1_root.T)
    out = leaky_relu(segsum(h1[src], dst) @ W2_rel.T + b2 + h1 @ W2_root.T + x)

Self-contained: takes full inputs, shards nodes across 8 NeuronCores internally,
runs one SPMD Bass program (gather/scatter-add DGE ops + PE matmuls + AllGather
halo exchange), and returns the full output.
"""
import sys

sys.path.insert(0, '/opt/trn_rl_repo')

import numpy as np

N = 100000
D = 64
NCORES = 8
NPART = N // NCORES            # 12500
NP = 12544                     # 98 * 128, padded part size
NCHUNK = NP // 128             # 98
DUMP = NP                      # first dump row in agg
G_CHUNK = 1024                 # idxs per gather/scatter call (= ring/2, baseline ratio)
AGG_ROWS = NP + G_CHUNK        # rows >= NP are per-call-unique dump rows
SCRATCH = 32768                # dynamic_dma_scratch_size (ring=2048 descs)
NSCAT = 6                      # round-robin scatter target tensors (hides WAW chains)
NEG_SLOPE = 0.01


def _round128(n):
    return ((n + 127) // 128) * 128


def _make_plan(src, dst):
    """Build the uniform SPMD call structure + per-core index streams.

    Edge stream order per core: by (q=src part, r=rank within (dst,q) group, dst).
    Per-(q,r) batch length = max over cores, rounded up to 128.
    Returns (plan, gstreams, sstreams):
      plan = dict(L, gcalls=[(q, start, len)], scalls=[(start, len)])
      gstreams/sstreams: int16 [NCORES, L] (gather idx into part-q tensor / agg row)
    """
    per_core = []
    maxr = 0
    for p in range(NCORES):
        sel = (dst >= p * NPART) & (dst < (p + 1) * NPART)
        s = src[sel]
        d = (dst[sel] - p * NPART).astype(np.int64)
        q = s // NPART
        sl = (s - q * NPART).astype(np.int64)
        o1 = np.lexsort((d, q))
        q, d, sl = q[o1], d[o1], sl[o1]
        key = q * NPART + d
        newgrp = np.r_[True, key[1:] != key[:-1]] if len(key) else np.zeros(0, bool)
        gid = np.cumsum(newgrp) - 1
        starts = np.flatnonzero(newgrp)
        r = np.arange(len(key)) - starts[gid] if len(key) else np.zeros(0, np.int64)
        maxr = max(maxr, int(r.max()) + 1 if len(r) else 0)
        per_core.append((q, r, d, sl))

    # counts[p, q, r]
    counts = np.zeros((NCORES, NCORES, maxr), np.int64)
    for p, (q, r, d, sl) in enumerate(per_core):
        np.add.at(counts, (p, q, r), 1)
    batch_len = np.zeros((NCORES, maxr), np.int64)
    for qq in range(NCORES):
        for rr in range(maxr):
            m = counts[:, qq, rr].max()
            if m > 0:
                batch_len[qq, rr] = _round128(m)

    # stream layout: q-major, r ascending
    batches = []  # (q, r, start, len)
    pos = 0
    for qq in range(NCORES):
        for rr in range(maxr):
            blen = int(batch_len[qq, rr])
            if blen:
                batches.append((qq, rr, pos, blen))
                pos += blen
    L = pos

    # gather calls: cut q-runs at G_CHUNK
    gcalls = []
    for qq in range(NCORES):
        qb = [b for b in batches if b[0] == qq]
        if not qb:
            continue
        q0, q1 = qb[0][2], qb[-1][2] + qb[-1][3]
        a = q0
        while a < q1:
            ln = min(G_CHUNK, q1 - a)
            gcalls.append((qq, a, ln))
            a += ln

    # scatter calls: breakpoints at batch starts + gcall starts, chop at G_CHUNK
    bks = sorted({b[2] for b in batches} | {g[1] for g in gcalls} | {L})
    scalls = []
    for i in range(len(bks) - 1):
        a, b = bks[i], bks[i + 1]
        while a < b:
            ln = min(G_CHUNK, b - a)
            scalls.append((a, ln))
            a += ln

    # slot permutation: agg row sigma(n) = 2048*b + K_b*p + k for n = 128*(16b+k)+p,
    # so p-major [128, K_b, 64] block loads land chunk-aligned
    nn = np.arange(NP)
    bb = nn // 2048
    kk = (nn % 2048) // 128
    pp = nn % 128
    Kb = np.minimum(16, NCHUNK - 16 * bb)
    sigma = 2048 * bb + Kb * pp + kk

    # per-core streams
    gstreams = np.zeros((NCORES, L), np.int16)
    sstreams = np.zeros((NCORES, L), np.int16)
    for p, (q, r, d, sl) in enumerate(per_core):
        gs = np.zeros(L, np.int64)
        ss = np.full(L, -1, np.int64)
        # edges of (q, r) batch placed at batch start, in d order (lexsort gives d asc
        # within (q, dst) groups -> within (q, r) also d asc)
        o2 = np.lexsort((d, r, q))
        q2, r2, d2, sl2 = q[o2], r[o2], d[o2], sl[o2]
        bstart = {(qq, rr): st for (qq, rr, st, ln) in batches}
        # offsets within each (q,r) batch: edges are sorted by (q,r,d); rank within
        # batch = position - first position of that batch
        key2 = q2 * maxr + r2
        nb = np.r_[True, key2[1:] != key2[:-1]] if len(key2) else np.zeros(0, bool)
        gid2 = np.cumsum(nb) - 1
        st2 = np.flatnonzero(nb)
        off = np.arange(len(key2)) - st2[gid2] if len(key2) else np.zeros(0, np.int64)
        base = np.array([bstart[(int(qq), int(rr))] for qq, rr in
                         zip(q2[st2], r2[st2])], np.int64) if len(st2) else np.zeros(0, np.int64)
        posn = base[gid2] + off
        gs[posn] = sl2
        ss[posn] = sigma[d2]
        # pads: scatter -> unique dump row per scall
        for (a, ln) in scalls:
            seg = ss[a:a + ln]
            pad = seg < 0
            seg[pad] = DUMP + np.flatnonzero(pad)
        gstreams[p] = gs.astype(np.int16)
        sstreams[p] = ss.astype(np.int16)

    plan = dict(L=L, gcalls=gcalls, scalls=scalls)
    return plan, gstreams, sstreams


def _wrap_stream(a):
    """[L] int16 -> [128, L//16] wrapped (idx i at [i%16, i//16]) replicated 8x."""
    L = len(a)
    assert L % 16 == 0
    w = a.reshape(L // 16, 16).T  # [16, cols]
    return np.tile(w, (8, 1)).copy()


def _build_nc(plan):
    from concourse import tile, mybir, masks
    import concourse.bacc as bacc

    L = plan["L"]
    cols = L // 16
    f32 = mybir.dt.float32
    i16 = mybir.dt.int16

    nc = bacc.Bacc(None, target_bir_lowering=False, num_devices=NCORES,
                   dynamic_dma_scratch_size=SCRATCH, num_swdge_queues=4)

    x_parts = [nc.declare_dram_parameter(f"x_part{q}", [NP, D], f32, isOutput=False)
               for q in range(NCORES)]
    xT_in = nc.declare_dram_parameter("xT", [D, NP], f32, isOutput=False)
    w_ins = {}
    for nm in ["W1relT", "W1rootT", "W2relT", "W2rootT"]:
        w_ins[nm] = nc.declare_dram_parameter(nm, [D, D], f32, isOutput=False)
    b_ins = {nm: nc.declare_dram_parameter(nm, [1, D], f32, isOutput=False)
             for nm in ["b1", "b2"]}
    gidx_in = nc.declare_dram_parameter("gidx", [128, cols], i16, isOutput=False)
    sidx_in = nc.declare_dram_parameter("sidx", [128, cols], i16, isOutput=False)
    y_out = nc.declare_dram_parameter("y", [NP, D], f32, isOutput=True)

    agg_a = [nc.dram_tensor(f"agg_a{t}", [AGG_ROWS, D], f32) for t in range(NSCAT)]
    agg_b = [nc.dram_tensor(f"agg_b{t}", [AGG_ROWS, D], f32) for t in range(NSCAT)]
    h1_bounce = nc.dram_tensor("h1_bounce", [NP, D], f32)
    h_full = nc.dram_tensor("h_full", [NCORES * NP, D], f32, addr_space="Shared")

    with tile.TileContext(nc) as tc:
        with (
            tc.tile_pool(name="const", bufs=1) as cpool,
            tc.tile_pool(name="idx", bufs=1) as ipool,
            tc.tile_pool(name="gbuf", bufs=8) as gpool,
            tc.tile_pool(name="mm", bufs=3) as mpool,
            tc.tile_pool(name="blk", bufs=2) as bpool,
            tc.tile_pool(name="psum", bufs=3, space="PSUM") as ppool,
        ):
            # ---- constants ----
            ident = cpool.tile([128, 128], f32)
            masks.make_identity(nc, ident[:])
            ones1 = cpool.tile([1, 128], f32)
            nc.gpsimd.memset(ones1[:], 1.0)
            wt = {}
            for nm, t_in in w_ins.items():
                t = cpool.tile([D, D], f32, tag=nm)
                nc.sync.dma_start(t[:], t_in[:])
                wt[nm] = t
            bt = {}
            for nm, t_in in b_ins.items():
                t = cpool.tile([1, D], f32, tag=nm)
                nc.sync.dma_start(t[:], t_in[:])
                bt[nm] = t

            # ---- zero both agg buffers ----
            ztile = cpool.tile([128, 16, D], f32)
            nc.gpsimd.memset(ztile[:], 0.0)
            for agg in agg_a + agg_b:
                for a in range(0, AGG_ROWS, 2048):
                    n = min(2048, AGG_ROWS - a)
                    nc.sync.dma_start(
                        agg[a:a + n, :].rearrange("(p k) d -> p k d", p=128),
                        ztile[:, :n // 128, :])

            # ---- index streams (resident; reused by both layers) ----
            gidx = ipool.tile([128, cols], i16)
            sidx = ipool.tile([128, cols], i16)
            for a in range(0, cols, 2048):
                n = min(2048, cols - a)
                nc.sync.dma_start(gidx[:, a:a + n], gidx_in[:, a:a + n])
                nc.sync.dma_start(sidx[:, a:a + n], sidx_in[:, a:a + n])

            # ---- gather + scatter-add layer ----
            def gs_layer(src_aps, aggs):
                si = 0
                gi_n = 0
                for (q, gstart, glen) in plan["gcalls"]:
                    rows = glen // 128
                    gb = gpool.tile([128, G_CHUNK // 128, D], f32, tag="gb")
                    nc.gpsimd.dma_gather(
                        gb[:, :rows, :], src_aps[q], gidx[:, gstart // 16:(gstart + glen) // 16],
                        glen, glen, D, queue_num=gi_n % 2)
                    gi_n += 1
                    for (sstart, slen) in plan["scalls"]:
                        if sstart < gstart or sstart >= gstart + glen:
                            continue
                        a = (sstart - gstart) // 128
                        b = a + slen // 128
                        nc.gpsimd.dma_scatter_add(
                            aggs[si % NSCAT][:], gb[:, a:b, :],
                            sidx[:, sstart // 16:(sstart + slen) // 16],
                            slen, slen, D, queue_num=2 + si % 2)
                        si += 1

            # ---- dense phase: h = lrelu(aggT.T@Wrel + rootT.T@Wroot [+ xT.T] + b) ----
            def dense_layer(aggs, w_rel, w_root, bias, root_rows, residual, out_rows):
                for blk in range((NCHUNK + 15) // 16):
                    K_b = min(16, NCHUNK - 16 * blk)
                    a0 = 2048 * blk
                    ablk = []
                    for t in range(NSCAT):
                        at = bpool.tile([128, 16, D], f32, tag=f"ablk{t}")
                        nc.sync.dma_start(
                            at[:, :K_b, :],
                            aggs[t][a0:a0 + 128 * K_b, :].rearrange(
                                "(p k) d -> p k d", p=128))
                        ablk.append(at)
                    xblk = bpool.tile([D, 2048], f32, tag="xblk")
                    nc.sync.dma_start(xblk[:, :128 * K_b],
                                      xT_in[:, a0:a0 + 128 * K_b])
                    for k in range(K_b):
                        c = 16 * blk + k
                        r0 = c * 128
                        ps_t = ppool.tile([D, 128], f32, tag="ps_t")
                        for t in range(NSCAT):
                            nc.tensor.matmul(ps_t[:], ablk[t][:, k, :], ident[:],
                                             is_transpose=True,
                                             start=(t == 0), stop=(t == NSCAT - 1))
                        aT = mpool.tile([D, 128], f32, tag="aT")
                        nc.vector.tensor_copy(aT[:], ps_t[:])

                        if root_rows is None:
                            rT = xblk[:, 128 * k:128 * (k + 1)]
                        else:
                            hc = mpool.tile([128, D], f32, tag="hc")
                            nc.sync.dma_start(hc[:], root_rows[r0:r0 + 128, :])
                            ps_h = ppool.tile([D, 128], f32, tag="ps_t")
                            nc.tensor.transpose(ps_h[:], hc[:], ident[:])
                            rTt = mpool.tile([D, 128], f32, tag="rT")
                            nc.vector.tensor_copy(rTt[:], ps_h[:])
                            rT = rTt[:]

                        po = ppool.tile([128, D], f32, tag="po")
                        nc.tensor.matmul(po[:], aT[:], w_rel[:], start=True, stop=False)
                        nc.tensor.matmul(po[:], rT, w_root[:], start=False, stop=False)
                        if residual:
                            nc.tensor.matmul(po[:], xblk[:, 128 * k:128 * (k + 1)],
                                             ident[:D, :D], start=False, stop=False)
                        nc.tensor.matmul(po[:], ones1[:], bias[:], start=False, stop=True)

                        tmp = mpool.tile([128, D], f32, tag="tmp")
                        nc.vector.tensor_scalar_mul(tmp[:], po[:], NEG_SLOPE)
                        hrow = mpool.tile([128, D], f32, tag="hrow")
                        nc.vector.tensor_max(hrow[:], po[:], tmp[:])
                        nc.sync.dma_start(out_rows[r0:r0 + 128, :], hrow[:])

            # ================= layer 1 =================
            gs_layer([xp[:] for xp in x_parts], agg_a)
            dense_layer(agg_a, wt["W1relT"], wt["W1rootT"], bt["b1"],
                        None, False, h1_bounce)

            # ================= halo exchange =================
            nc.gpsimd.collective_compute(
                "AllGather", mybir.AluOpType.bypass,
                replica_groups=[list(range(NCORES))],
                ins=[h1_bounce[:].opt()], outs=[h_full[:].opt()])

            # ================= layer 2 =================
            gs_layer([h_full[q * NP:(q + 1) * NP, :] for q in range(NCORES)], agg_b)
            dense_layer(agg_b, wt["W2relT"], wt["W2rootT"], bt["b2"],
                        h1_bounce, True, y_out)

    nc.compile()
    return nc


def _prep_inputs(x, edge_index, W1_rel, b1, W1_root, W2_rel, b2, W2_root):
    src = np.asarray(edge_index[0]).astype(np.int64)
    dst = np.asarray(edge_index[1]).astype(np.int64)
    plan, gstreams, sstreams = _make_plan(src, dst)

    x = np.asarray(x, np.float32)
    xp_all = []
    for q in range(NCORES):
        xp = np.zeros((NP, D), np.float32)
        xp[:NPART] = x[q * NPART:(q + 1) * NPART]
        xp_all.append(xp)

    common = {f"x_part{q}": xp_all[q] for q in range(NCORES)}
    common["W1relT"] = np.ascontiguousarray(np.asarray(W1_rel, np.float32).T)
    common["W1rootT"] = np.ascontiguousarray(np.asarray(W1_root, np.float32).T)
    common["W2relT"] = np.ascontiguousarray(np.asarray(W2_rel, np.float32).T)
    common["W2rootT"] = np.ascontiguousarray(np.asarray(W2_root, np.float32).T)
    common["b1"] = np.asarray(b1, np.float32).reshape(1, D)
    common["b2"] = np.asarray(b2, np.float32).reshape(1, D)

    in_maps = []
    for p in range(NCORES):
        m = dict(common)
        m["xT"] = np.ascontiguousarray(xp_all[p].T)
        m["gidx"] = _wrap_stream(gstreams[p])
        m["sidx"] = _wrap_stream(sstreams[p])
        in_maps.append(m)
    return plan, in_maps


def kernel(x, edge_index, W1_rel, b1, W1_root, W2_rel, b2, W2_root):
    from concourse import bass_utils

    plan, in_maps = _prep_inputs(x, edge_index, W1_rel, b1, W1_root,
                                 W2_rel, b2, W2_root)
    nc = _build_nc(plan)
    res = bass_utils.run_bass_kernel_spmd(nc, in_maps, core_ids=list(range(NCORES)))
    out = np.concatenate([res.results[p]["y"][:NPART] for p in range(NCORES)], 0)
    return out.astype(np.float32)


if __name__ == "__main__":
    # quick host-side plan self-check in numpy (no device)
    rng = np.random.default_rng(0)
    E = 200000
    src = rng.integers(0, N, E)
    dst = rng.integers(0, N, E)
    plan, gstreams, sstreams = _make_plan(src, dst)
    print(f"L={plan['L']} gcalls={len(plan['gcalls'])} scalls={len(plan['scalls'])}")
    # emulate per-core layer-1 aggregation and compare against direct segment sum
    x = rng.normal(size=(N, D)).astype(np.float32)
    for p in range(2):
        agg = np.zeros((AGG_ROWS, D), np.float64)
        gs, ss = gstreams[p].astype(np.int64), sstreams[p].astype(np.int64)
        for (q, a, ln) in plan["gcalls"]:
            xq = np.zeros((NP, D), np.float32)
            xq[:NPART] = x[q * NPART:(q + 1) * NPART]
            g = xq[gs[a:a + ln]]
            for (sa, sl) in plan["scalls"]:
                if sa < a or sa >= a + ln:
                    continue
                seg = ss[sa:sa + sl]
                assert len(np.unique(seg)) == len(seg), "dup dst in scall!"
                np.add.at(agg, seg, g[sa - a:sa - a + sl])
        sel = (dst >= p * NPART) & (dst < (p + 1) * NPART)
        ref = np.zeros((NPART, D), np.float64)
        np.add.at(ref, dst[sel] - p * NPART, x[src[sel]])
        err = np.abs(agg[:NPART] - ref).max()
        print(f"core {p}: plan-emulated agg err {err:.3e}")



# revision 2
# speedup vs baseline: 1.3559x; 1.3559x over previous
"""Trainium2 Bass kernel for a 2-layer GraphConv block (PyG GraphConv, aggr=add):
    h1  = leaky_relu(segsum(x[src], dst) @ W1_rel.T + b1 + x @ W1_root.T)
    out = leaky_relu(segsum(h1[src], dst) @ W2_rel.T + b2 + h1 @ W2_root.T + x)

Source-stationary SPMD design (no DRAM scatter-add):
  - Core p owns edges with src in node-part p and gathers ONLY from its local
    x/h1 part (random 256B reads).
  - Edges sorted by (padded) dst; dsts fall into 128-row windows. Per chunk of
    up-to-128 gathered rows, a one-hot matrix (DVE is_equal vs iota)
    scatter-adds them into a per-window PSUM accumulator via one PE matmul.
  - Window partials staged to DRAM (bf16). Windows are processed in 7
    round-robin passes over the 8 node stripes, so each pass's partials can
    ReduceScatter(add) while later passes still accumulate; the dense phase
    (weights, bias, residual, leaky-relu) trickles along one pass behind.
  - Layer transitions stay local: core p's dense output h1 is exactly the
    gather source it needs for layer 2.
"""
import sys

sys.path.insert(0, '/opt/trn_rl_repo')

import numpy as np
import ml_dtypes

BF16 = ml_dtypes.bfloat16

N = 100000
D = 64
NCORES = 8
NPART = N // NCORES            # 12500 real nodes per stripe
NP = 12544                     # 98*128 padded stripe rows
NW = (NP * NCORES) // 128      # 784 global 128-row dst windows
WPS = NP // 128                # 98 windows per stripe
GROUP = 7                      # windows per PSUM accumulator tile (1792B bank)
SLAB = 14                      # windows per stage DMA slab (2 groups)
NPASS = 7                      # slab round-robin passes (slab s in pass s%7)
WPP = NW // NPASS              # 112 windows per pass
CALL_COLS = 8                  # 128-row chunks per dma_gather call (1024 idxs)
SCRATCH = 32768                # SWDGE ring: 2048 descriptors (SBUF bytes/partition)
NEG_SLOPE = 0.01
SENT = 255.0                   # dlow sentinel (outside 0..127, bf16-exact)
CAP128 = 128                   # nodes per window


def _pack_stripe(deg, nbig=8, light_cap=254, big_cap=500):
    """Assign a stripe's NPART nodes to its 98 windows (<=128 nodes each),
    minimizing ceil(max_core_load/128) per window: most windows stay under
    2 chunks, a few designated big windows absorb the heavy nodes."""
    tot = deg.sum(1)
    order = np.argsort(-tot, kind="stable")
    loads = np.zeros((WPS, NCORES), np.int64)
    cnt = np.zeros(WPS, np.int64)
    win = np.full(NPART, -1, np.int32)
    BIG = 10 ** 9
    caps = np.full(WPS, light_cap)
    caps[WPS - nbig:] = big_cap
    for n in order:
        d = deg[n]
        new = loads + d
        newmax = new.max(1)
        ok = (newmax <= caps) & (cnt < CAP128)
        if ok.any():
            score = np.where(ok, newmax.astype(np.float64) / caps, BIG)
        else:
            score = np.where(cnt < CAP128, newmax.astype(np.float64), BIG)
        w = int(np.argmin(score))
        win[n] = w
        loads[w] += d
        cnt[w] += 1
    return win, loads, cnt


def _make_plan(src, dst):
    """Uniform SPMD schedule + per-core gather/dst-low streams.

    Nodes are re-packed into windows per stripe (see _pack_stripe); the
    resulting per-stripe permutation is applied host-side to x_part/xT/y and
    to the gather indices, so the device only ever sees the packed layout.
    Edges of core p = edges with src in part p, sorted by (packed) dst
    window; window w gets nch[w] = max_p ceil(count_pw/128) 128-edge chunks;
    per-core edges pack at the window start, rest is pad (gidx 0, dlow
    sentinel -> zero one-hot row contributes nothing).
    Windows are laid out in slab round-robin order: pass k processes slab
    7*j + k of every stripe j.
    """
    psrc = src // NPART
    dstq = dst // NPART
    dstl = dst % NPART
    plocal = np.empty((NCORES, NPART), np.int64)
    loads_all = np.zeros((NCORES, WPS, NCORES), np.int64)
    for q in range(NCORES):
        sel = dstq == q
        deg = np.zeros((NPART, NCORES), np.int64)
        np.add.at(deg, (dstl[sel], psrc[sel]), 1)
        win, loads, cnt = _pack_stripe(deg)
        # relabel windows so chunk counts balance across the 7 slabs
        # (pass k of the round-robin processes slab k of every stripe)
        nchw = np.maximum((loads.max(1) + 127) // 128, 1)
        order = np.argsort(-nchw, kind="stable")
        relabel = np.empty(WPS, np.int64)
        slot_in_slab = np.zeros(NPASS, np.int64)
        si = 0
        for i, wold in enumerate(order):
            while slot_in_slab[si % NPASS] >= SLAB:
                si += 1
            s = si % NPASS
            relabel[wold] = s * SLAB + slot_in_slab[s]
            slot_in_slab[s] += 1
            si += 1
        win = relabel[win].astype(np.int32)
        loads2 = np.zeros((WPS, NCORES), np.int64)
        np.add.at(loads2, (win[dstl[sel]], psrc[sel]), 1)
        slot = np.zeros(NPART, np.int64)
        for w in range(WPS):
            nodes = np.flatnonzero(win == w)
            slot[nodes] = np.arange(len(nodes))
        plocal[q] = win.astype(np.int64) * 128 + slot
        loads_all[q] = loads2

    dpad = dstq * NP + plocal[dstq, dstl]
    mx = loads_all.max(2).reshape(NCORES * WPS)  # [NW] in stripe-major order
    nch = np.maximum((mx + 127) // 128, 1)

    worder = [w
              for k in range(NPASS)
              for j in range(NCORES)
              for w in range(SLAB * (NPASS * j + k), SLAB * (NPASS * j + k) + SLAB)]
    worder = np.array(worder)
    C = int(nch.sum())
    coff = np.zeros(NW, np.int64)
    coff[worder] = np.cumsum(nch[worder]) - nch[worder]
    L = 128 * C
    sched = [(int(w), int(coff[w]), int(nch[w])) for w in worder]

    gstreams = np.zeros((NCORES, L), np.int16)
    dstreams = np.empty((NCORES, L), np.float64)
    for p in range(NCORES):
        sel = psrc == p
        dp = dpad[sel]
        sp = plocal[p][src[sel] % NPART]
        o = np.argsort(dp, kind="stable")
        dp, sp = dp[o], sp[o]
        wv = dp >> 7
        new = np.r_[True, wv[1:] != wv[:-1]]
        st = np.flatnonzero(new)
        gid = np.cumsum(new) - 1
        rank = np.arange(len(dp)) - st[gid]
        pos = coff[wv] * 128 + rank
        gs = np.zeros(L, np.int64)
        gs[pos] = sp
        ds = np.full(L, SENT, np.float64)
        ds[pos] = dp & 127
        gstreams[p] = gs.astype(np.int16)
        dstreams[p] = ds

    return dict(C=C, L=L, sched=sched), gstreams, dstreams, plocal


def _wrap_stream(a):
    """[L] int16 -> [128, L//16] wrapped (idx i at [i%16, i//16]) replicated 8x."""
    L = len(a)
    assert L % 16 == 0
    w = a.reshape(L // 16, 16).T
    return np.tile(w, (8, 1)).copy()


def _build_nc(plan):
    from concourse import tile, mybir
    import concourse.bacc as bacc

    C = plan["C"]
    sched = plan["sched"]
    f32 = mybir.dt.float32
    bf16 = mybir.dt.bfloat16
    i16 = mybir.dt.int16
    AF = mybir.ActivationFunctionType
    Alu = mybir.AluOpType

    nc = bacc.Bacc(None, target_bir_lowering=False, num_devices=NCORES,
                   dynamic_dma_scratch_size=SCRATCH, num_swdge_queues=4)

    x_part = nc.declare_dram_parameter("x_part", [NP, D], f32, isOutput=False)
    xT_in = nc.declare_dram_parameter("xT", [D, NP], bf16, isOutput=False)
    w_ins = {nm: nc.declare_dram_parameter(nm, [D, D], bf16, isOutput=False)
             for nm in ["W1relT", "W1rootT", "W2relT", "W2rootT"]}
    b_ins = {nm: nc.declare_dram_parameter(nm, [1, D], bf16, isOutput=False)
             for nm in ["b1", "b2"]}
    id64_in = nc.declare_dram_parameter("id64", [D, D], bf16, isOutput=False)
    idb_in = nc.declare_dram_parameter("idb", [128, 128], bf16, isOutput=False)
    idf_in = nc.declare_dram_parameter("idf", [128, 128], f32, isOutput=False)
    # d-major iota: iota[p, d*CALL_COLS + c] = d (packed along c for 2x DVE)
    iota_in = nc.declare_dram_parameter("iota", [128, 128 * CALL_COLS], bf16,
                                        isOutput=False)
    gidx_in = nc.declare_dram_parameter("gidx", [128, C * 8], i16, isOutput=False)
    dlow_in = nc.declare_dram_parameter("dlow", [128, C], bf16, isOutput=False)
    y_out = nc.declare_dram_parameter("y", [NP, D], f32, isOutput=True)

    partials = [[nc.dram_tensor(f"partial{l}_{k}", [NCORES * SLAB * 128, D], bf16)
                 for k in range(NPASS)] for l in range(2)]
    exchs = [[nc.dram_tensor(f"exch{l}_{k}", [NCORES * SLAB * 128, D], bf16)
              for k in range(NPASS)] for l in range(2)]
    h1 = nc.dram_tensor("h1", [NP, D], f32)

    with tile.TileContext(nc) as tc:
        with (
            tc.tile_pool(name="const", bufs=1) as cpool,
            tc.tile_pool(name="idx", bufs=1) as ipool,
            tc.tile_pool(name="gb", bufs=4) as gpool,
            tc.tile_pool(name="gbh", bufs=4) as hpool,
            tc.tile_pool(name="oh", bufs=4) as opool,
            tc.tile_pool(name="stage", bufs=2) as spool,
            tc.tile_pool(name="dense", bufs=2) as dpool,
            tc.tile_pool(name="mm", bufs=3) as mpool,
            tc.tile_pool(name="pacc", bufs=3, space="PSUM") as ppool,
            tc.tile_pool(name="pt", bufs=1, space="PSUM") as tpool,
            tc.tile_pool(name="po", bufs=2, space="PSUM") as qpool,
        ):
            # ---- constants ----
            wt = {}
            for nm, t_in in w_ins.items():
                t = cpool.tile([D, D], bf16, tag=nm)
                nc.sync.dma_start(t[:], t_in[:])
                wt[nm] = t
            bt = {}
            for nm, t_in in b_ins.items():
                t = cpool.tile([1, D], bf16, tag=nm)
                nc.sync.dma_start(t[:], t_in[:])
                bt[nm] = t
            id64 = cpool.tile([D, D], bf16, tag="id64")
            nc.sync.dma_start(id64[:], id64_in[:])
            identb = cpool.tile([128, 128], bf16, tag="idb")
            nc.sync.dma_start(identb[:], idb_in[:])
            identf = cpool.tile([128, 128], f32, tag="idf")
            nc.sync.dma_start(identf[:], idf_in[:])
            iota_t = cpool.tile([128, 128 * CALL_COLS], bf16, tag="iota")
            nc.sync.dma_start(iota_t[:], iota_in[:])
            ones1 = cpool.tile([1, 128], bf16, tag="ones1")
            nc.gpsimd.memset(ones1[:], 1.0)
            xT_t = cpool.tile([D, NP], bf16, tag="xT")
            for a in range(0, NP, 4096):
                n = min(4096, NP - a)
                nc.sync.dma_start(xT_t[:, a:a + n], xT_in[:, a:a + n])
            gidx = ipool.tile([128, C * 8], i16)
            for a in range(0, C * 8, 2048):
                n = min(2048, C * 8 - a)
                nc.sync.dma_start(gidx[:, a:a + n], gidx_in[:, a:a + n])
            dlow_t = ipool.tile([128, C], bf16)
            nc.sync.dma_start(dlow_t[:], dlow_in[:])

            # ---- gather + one-hot PE scatter-add into window partials ----
            def make_accum(src_dram, parts):
                tiles = {}
                state = {"next": 0}

                def ensure_call(col):
                    while col >= state["next"] * CALL_COLS:
                        g = state["next"]
                        c0 = g * CALL_COLS
                        n = min(CALL_COLS, C - c0)
                        gb = gpool.tile([128, CALL_COLS, D], f32, tag="gb")
                        nc.gpsimd.dma_gather(
                            gb[:, :n, :], src_dram, gidx[:, c0 * 8:(c0 + n) * 8],
                            n * 128, n * 128, D, queue_num=g % 2)
                        gbh = hpool.tile([128, CALL_COLS, D], bf16, tag="gbh")
                        nc.vector.tensor_copy(gbh[:, :n, :], gb[:, :n, :])
                        oh = opool.tile([128, 128, CALL_COLS], bf16, tag="oh")
                        nc.vector.tensor_tensor(
                            oh[:, :, :n],
                            iota_t[:, :].rearrange(
                                "p (d c) -> p d c", c=CALL_COLS)[:, :, :n],
                            dlow_t[:, c0:c0 + n].unsqueeze(1).broadcast_to(
                                (128, 128, n)),
                            Alu.is_equal)
                        for cc in range(c0, c0 + n):
                            tiles[cc] = (oh, gbh, c0)
                        state["next"] += 1

                def accum_pass(k, pending_a2a=None):
                    sg = None
                    for (w, c0, nchw) in sched[WPP * k:WPP * (k + 1)]:
                        if pending_a2a is not None and w % GROUP == GROUP - 1:
                            # dispatch the previous pass's collective right
                            # after this pass's first desc-gens are queued
                            pending_a2a()
                            pending_a2a = None
                        if w % GROUP == 0:
                            pg = ppool.tile([128, GROUP, D], f32, tag="pg")
                        else:
                            pg = state["pg"]
                        state["pg"] = pg
                        ensure_call(c0 + nchw - 1)
                        for i in range(nchw):
                            col = c0 + i
                            oh, gbh, cb = tiles[col]
                            nc.tensor.matmul(
                                pg[:, w % GROUP, :],
                                oh[:, :, col - cb],
                                gbh[:, col - cb, :],
                                start=(i == 0), stop=(i == nchw - 1))
                        if w % GROUP == GROUP - 1:
                            gi = (w % SLAB) // GROUP
                            if gi == 0:
                                sg = spool.tile([128, SLAB, D], bf16, tag="sg")
                            nc.scalar.activation(
                                sg[:, gi * GROUP:(gi + 1) * GROUP, :],
                                pg[:, :, :], AF.Copy)
                            if gi == 1:
                                s = w // SLAB
                                jj = s // NPASS
                                nc.sync.dma_start(
                                    parts[s % NPASS][
                                        1792 * jj:1792 * (jj + 1), :].rearrange(
                                        "(p k) d -> p k d", p=128),
                                    sg[:, :, :])

                return accum_pass

            def a2a_pass(parts, exch, k):
                nc.gpsimd.collective_compute(
                    "AllToAll", mybir.AluOpType.bypass,
                    replica_groups=[list(range(NCORES))],
                    ins=[parts[k][:].opt()],
                    outs=[exch[k][:].opt()])

            # ---- dense: h = lrelu(aggT.T@WrelT + rootT.T@WrootT [+x] + b) ----
            # The 8 senders' partials are summed on the PE: the per-window
            # transpose accumulates all 8 exchanged slabs in PSUM.
            def dense_pass(exch, wrel, wroot, bias_t, root_dram, residual,
                           out_dram, k):
                ex = dpool.tile([128, NCORES, SLAB, D], bf16, tag="ex")
                nc.scalar.dma_start(
                    ex[:],
                    exch[k][:].rearrange("(j p k) d -> p j k d", p=128, k=SLAB))
                hs = None
                if root_dram is not None:
                    hs = dpool.tile([128, SLAB, D], f32, tag="hs")
                    nc.scalar.dma_start(
                        hs[:],
                        root_dram[1792 * k:1792 * (k + 1), :].rearrange(
                            "(k p) d -> p k d", k=SLAB))
                for kk in range(SLAB):
                    r0 = 1792 * k + 128 * kk
                    pt = tpool.tile([D, 128], f32, tag="pt")
                    for j in range(NCORES):
                        # out[d, n] = sum_e ex[e, d] * I[e, n]  (transpose-acc)
                        nc.tensor.matmul(pt[:], ex[:, j, kk, :], identb[:],
                                         start=(j == 0), stop=(j == NCORES - 1))
                    aT = mpool.tile([D, 128], bf16, tag="aT")
                    nc.scalar.activation(aT[:], pt[:], AF.Copy)
                    if root_dram is not None:
                        pt2 = tpool.tile([D, 128], f32, tag="ptf")
                        nc.tensor.matmul(pt2[:], hs[:, kk, :], identf[:],
                                         start=True, stop=True)
                        rT = mpool.tile([D, 128], bf16, tag="rT")
                        nc.scalar.activation(rT[:], pt2[:], AF.Copy)
                        rTap = rT[:]
                    else:
                        rTap = xT_t[:, r0:r0 + 128]
                    po = qpool.tile([128, D], f32, tag="po")
                    nc.tensor.matmul(po[:], aT[:], wrel[:],
                                     start=True, stop=False)
                    nc.tensor.matmul(po[:], rTap, wroot[:],
                                     start=False, stop=False)
                    if residual:
                        nc.tensor.matmul(po[:], xT_t[:, r0:r0 + 128],
                                         id64[:], start=False, stop=False)
                    nc.tensor.matmul(po[:], ones1[:], bias_t[:],
                                     start=False, stop=True)
                    hrow = mpool.tile([128, D], f32, tag="hrow")
                    nc.scalar.activation(hrow[:], po[:], AF.Lrelu,
                                         alpha=NEG_SLOPE)
                    nc.scalar.dma_start(out_dram[r0:r0 + 128, :], hrow[:])

            # ---- layer driver: lag the collective by 1 pass and dense by 2
            # passes so in-order engine queues never head-of-line block.
            def layer(src_dram, parts, exch, wrel, wroot, bias_t, root_dram,
                      residual, out_dram):
                accum_pass = make_accum(src_dram, parts)
                for k in range(NPASS):
                    pend = None
                    if k >= 1:
                        pend = (lambda kk: lambda: a2a_pass(parts, exch, kk))(k - 1)
                    accum_pass(k, pend)
                    if k >= 2:
                        dense_pass(exch, wrel, wroot, bias_t, root_dram,
                                   residual, out_dram, k - 2)
                dense_pass(exch, wrel, wroot, bias_t, root_dram, residual,
                           out_dram, NPASS - 2)
                a2a_pass(parts, exch, NPASS - 1)
                dense_pass(exch, wrel, wroot, bias_t, root_dram, residual,
                           out_dram, NPASS - 1)

            layer(x_part[:], partials[0], exchs[0], wt["W1relT"], wt["W1rootT"],
                  bt["b1"], None, False, h1)
            layer(h1[:], partials[1], exchs[1], wt["W2relT"], wt["W2rootT"],
                  bt["b2"], h1, True, y_out)

    nc.compile()
    return nc


def _prep_inputs(x, edge_index, W1_rel, b1, W1_root, W2_rel, b2, W2_root):
    src = np.asarray(edge_index[0]).astype(np.int64)
    dst = np.asarray(edge_index[1]).astype(np.int64)
    plan, gstreams, dstreams, plocal = _make_plan(src, dst)
    plan["plocal"] = plocal
    C = plan["C"]

    x = np.asarray(x, np.float32)
    iota = np.tile(np.repeat(np.arange(128, dtype=np.float32), CALL_COLS),
                   (128, 1)).astype(BF16)
    id64 = np.eye(D, dtype=np.float32).astype(BF16)
    idb = np.eye(128, dtype=np.float32).astype(BF16)
    idf = np.eye(128, dtype=np.float32)

    common = {
        "W1relT": np.ascontiguousarray(np.asarray(W1_rel, np.float32).T).astype(BF16),
        "W1rootT": np.ascontiguousarray(np.asarray(W1_root, np.float32).T).astype(BF16),
        "W2relT": np.ascontiguousarray(np.asarray(W2_rel, np.float32).T).astype(BF16),
        "W2rootT": np.ascontiguousarray(np.asarray(W2_root, np.float32).T).astype(BF16),
        "b1": np.asarray(b1, np.float32).reshape(1, D).astype(BF16),
        "b2": np.asarray(b2, np.float32).reshape(1, D).astype(BF16),
        "id64": id64, "idb": idb, "idf": idf, "iota": iota,
    }

    in_maps = []
    for p in range(NCORES):
        xp = np.zeros((NP, D), np.float32)
        xp[plocal[p]] = x[p * NPART:(p + 1) * NPART]
        m = dict(common)
        m["x_part"] = xp
        m["xT"] = np.ascontiguousarray(xp.T).astype(BF16)
        m["gidx"] = _wrap_stream(gstreams[p])
        m["dlow"] = np.ascontiguousarray(
            dstreams[p].reshape(C, 128).T).astype(BF16)
        in_maps.append(m)
    return plan, in_maps


def kernel(x, edge_index, W1_rel, b1, W1_root, W2_rel, b2, W2_root):
    from concourse import bass_utils

    plan, in_maps = _prep_inputs(x, edge_index, W1_rel, b1, W1_root,
                                 W2_rel, b2, W2_root)
    nc = _build_nc(plan)
    res = bass_utils.run_bass_kernel_spmd(nc, in_maps, core_ids=list(range(NCORES)))
    plocal = plan["plocal"]
    out = np.concatenate(
        [res.results[p]["y"][plocal[p]] for p in range(NCORES)], 0)
    return out.astype(np.float32)


# revision 3
# speedup vs baseline: 1.3594x; 1.0026x over previous
"""Trainium2 Bass kernel for a 2-layer GraphConv block (PyG GraphConv, aggr=add):
    h1  = leaky_relu(segsum(x[src], dst) @ W1_rel.T + b1 + x @ W1_root.T)
    out = leaky_relu(segsum(h1[src], dst) @ W2_rel.T + b2 + h1 @ W2_root.T + x)

Source-stationary SPMD design (no DRAM scatter-add):
  - Core p owns edges with src in node-part p and gathers ONLY from its local
    x/h1 part (random 256B reads).
  - Edges sorted by (padded) dst; dsts fall into 128-row windows. Per chunk of
    up-to-128 gathered rows, a one-hot matrix (DVE is_equal vs iota)
    scatter-adds them into a per-window PSUM accumulator via one PE matmul.
  - Window partials staged to DRAM (bf16). Windows are processed in 7
    round-robin passes over the 8 node stripes, so each pass's partials can
    ReduceScatter(add) while later passes still accumulate; the dense phase
    (weights, bias, residual, leaky-relu) trickles along one pass behind.
  - Layer transitions stay local: core p's dense output h1 is exactly the
    gather source it needs for layer 2.
"""
import sys

sys.path.insert(0, '/opt/trn_rl_repo')

import numpy as np
import ml_dtypes

BF16 = ml_dtypes.bfloat16

N = 100000
D = 64
NCORES = 8
NPART = N // NCORES            # 12500 real nodes per stripe
NP = 12544                     # 98*128 padded stripe rows
NW = (NP * NCORES) // 128      # 784 global 128-row dst windows
WPS = NP // 128                # 98 windows per stripe
GROUP = 7                      # windows per PSUM accumulator tile (1792B bank)
SLAB = 14                      # windows per stage DMA slab (2 groups)
NPASS = 7                      # slab round-robin passes (slab s in pass s%7)
WPP = NW // NPASS              # 112 windows per pass
CALL_COLS = 8                  # 128-row chunks per dma_gather call (1024 idxs)
SCRATCH = 32768                # SWDGE ring: 2048 descriptors (SBUF bytes/partition)
NEG_SLOPE = 0.01
SENT = 255.0                   # dlow sentinel (outside 0..127, bf16-exact)
CAP128 = 128                   # nodes per window
DLAG = 3                       # dense lags accum by DLAG passes (hides a2a)


def _pack_stripe(deg, nbig=8, light_cap=254, big_cap=500):
    """Assign a stripe's NPART nodes to its 98 windows (<=128 nodes each),
    minimizing ceil(max_core_load/128) per window: most windows stay under
    2 chunks, a few designated big windows absorb the heavy nodes."""
    tot = deg.sum(1)
    order = np.argsort(-tot, kind="stable")
    loads = np.zeros((WPS, NCORES), np.int64)
    cnt = np.zeros(WPS, np.int64)
    win = np.full(NPART, -1, np.int32)
    BIG = 10 ** 9
    caps = np.full(WPS, light_cap)
    caps[WPS - nbig:] = big_cap
    for n in order:
        d = deg[n]
        new = loads + d
        newmax = new.max(1)
        ok = (newmax <= caps) & (cnt < CAP128)
        if ok.any():
            score = np.where(ok, newmax.astype(np.float64) / caps, BIG)
        else:
            score = np.where(cnt < CAP128, newmax.astype(np.float64), BIG)
        w = int(np.argmin(score))
        win[n] = w
        loads[w] += d
        cnt[w] += 1
    return win, loads, cnt


def _make_plan(src, dst):
    """Uniform SPMD schedule + per-core gather/dst-low streams.

    Nodes are re-packed into windows per stripe (see _pack_stripe); the
    resulting per-stripe permutation is applied host-side to x_part/xT/y and
    to the gather indices, so the device only ever sees the packed layout.
    Edges of core p = edges with src in part p, sorted by (packed) dst
    window; window w gets nch[w] = max_p ceil(count_pw/128) 128-edge chunks;
    per-core edges pack at the window start, rest is pad (gidx 0, dlow
    sentinel -> zero one-hot row contributes nothing).
    Windows are laid out in slab round-robin order: pass k processes slab
    7*j + k of every stripe j.
    """
    psrc = src // NPART
    dstq = dst // NPART
    dstl = dst % NPART
    plocal = np.empty((NCORES, NPART), np.int64)
    loads_all = np.zeros((NCORES, WPS, NCORES), np.int64)
    for q in range(NCORES):
        sel = dstq == q
        deg = np.zeros((NPART, NCORES), np.int64)
        np.add.at(deg, (dstl[sel], psrc[sel]), 1)
        win, loads, cnt = _pack_stripe(deg)
        # relabel windows so chunk counts balance across the 7 slabs
        # (pass k of the round-robin processes slab k of every stripe)
        nchw = np.maximum((loads.max(1) + 127) // 128, 1)
        order = np.argsort(-nchw, kind="stable")
        relabel = np.empty(WPS, np.int64)
        slot_in_slab = np.zeros(NPASS, np.int64)
        si = 0
        for i, wold in enumerate(order):
            while slot_in_slab[si % NPASS] >= SLAB:
                si += 1
            s = si % NPASS
            relabel[wold] = s * SLAB + slot_in_slab[s]
            slot_in_slab[s] += 1
            si += 1
        win = relabel[win].astype(np.int32)
        loads2 = np.zeros((WPS, NCORES), np.int64)
        np.add.at(loads2, (win[dstl[sel]], psrc[sel]), 1)
        slot = np.zeros(NPART, np.int64)
        for w in range(WPS):
            nodes = np.flatnonzero(win == w)
            slot[nodes] = np.arange(len(nodes))
        plocal[q] = win.astype(np.int64) * 128 + slot
        loads_all[q] = loads2

    dpad = dstq * NP + plocal[dstq, dstl]
    mx = loads_all.max(2).reshape(NCORES * WPS)  # [NW] in stripe-major order
    nch = np.maximum((mx + 127) // 128, 1)

    worder = [w
              for k in range(NPASS)
              for j in range(NCORES)
              for w in range(SLAB * (NPASS * j + k), SLAB * (NPASS * j + k) + SLAB)]
    worder = np.array(worder)
    C = int(nch.sum())
    coff = np.zeros(NW, np.int64)
    coff[worder] = np.cumsum(nch[worder]) - nch[worder]
    L = 128 * C
    sched = [(int(w), int(coff[w]), int(nch[w])) for w in worder]

    gstreams = np.zeros((NCORES, L), np.int16)
    dstreams = np.empty((NCORES, L), np.float64)
    for p in range(NCORES):
        sel = psrc == p
        dp = dpad[sel]
        sp = plocal[p][src[sel] % NPART]
        o = np.argsort(dp, kind="stable")
        dp, sp = dp[o], sp[o]
        wv = dp >> 7
        new = np.r_[True, wv[1:] != wv[:-1]]
        st = np.flatnonzero(new)
        gid = np.cumsum(new) - 1
        rank = np.arange(len(dp)) - st[gid]
        pos = coff[wv] * 128 + rank
        gs = np.zeros(L, np.int64)
        gs[pos] = sp
        ds = np.full(L, SENT, np.float64)
        ds[pos] = dp & 127
        gstreams[p] = gs.astype(np.int16)
        dstreams[p] = ds

    return dict(C=C, L=L, sched=sched), gstreams, dstreams, plocal


def _wrap_stream(a):
    """[L] int16 -> [128, L//16] wrapped (idx i at [i%16, i//16]) replicated 8x."""
    L = len(a)
    assert L % 16 == 0
    w = a.reshape(L // 16, 16).T
    return np.tile(w, (8, 1)).copy()


def _build_nc(plan):
    from concourse import tile, mybir
    import concourse.bacc as bacc

    C = plan["C"]
    sched = plan["sched"]
    f32 = mybir.dt.float32
    bf16 = mybir.dt.bfloat16
    i16 = mybir.dt.int16
    AF = mybir.ActivationFunctionType
    Alu = mybir.AluOpType

    nc = bacc.Bacc(None, target_bir_lowering=False, num_devices=NCORES,
                   dynamic_dma_scratch_size=SCRATCH, num_swdge_queues=4)

    x_part = nc.declare_dram_parameter("x_part", [NP, D], f32, isOutput=False)
    xT_in = nc.declare_dram_parameter("xT", [D, NP], bf16, isOutput=False)
    w_ins = {nm: nc.declare_dram_parameter(nm, [D, D], bf16, isOutput=False)
             for nm in ["W1relT", "W1rootT", "W2relT", "W2rootT"]}
    b_ins = {nm: nc.declare_dram_parameter(nm, [1, D], bf16, isOutput=False)
             for nm in ["b1", "b2"]}
    id64_in = nc.declare_dram_parameter("id64", [D, D], bf16, isOutput=False)
    idb_in = nc.declare_dram_parameter("idb", [128, 128], bf16, isOutput=False)
    idf_in = nc.declare_dram_parameter("idf", [128, 128], f32, isOutput=False)
    # d-major iota: iota[p, d*CALL_COLS + c] = d (packed along c for 2x DVE)
    iota_in = nc.declare_dram_parameter("iota", [128, 128 * CALL_COLS], bf16,
                                        isOutput=False)
    gidx_in = nc.declare_dram_parameter("gidx", [128, C * 8], i16, isOutput=False)
    dlow_in = nc.declare_dram_parameter("dlow", [128, C], bf16, isOutput=False)
    y_out = nc.declare_dram_parameter("y", [NP, D], f32, isOutput=True)

    partials = [[nc.dram_tensor(f"partial{l}_{k}", [NCORES * SLAB * 128, D], bf16)
                 for k in range(NPASS)] for l in range(2)]
    exchs = [[nc.dram_tensor(f"exch{l}_{k}", [NCORES * SLAB * 128, D], bf16)
              for k in range(NPASS)] for l in range(2)]
    h1 = nc.dram_tensor("h1", [NP, D], f32)

    with tile.TileContext(nc) as tc:
        with (
            tc.tile_pool(name="const", bufs=1) as cpool,
            tc.tile_pool(name="idx", bufs=1) as ipool,
            tc.tile_pool(name="gb", bufs=4) as gpool,
            tc.tile_pool(name="gbh", bufs=4) as hpool,
            tc.tile_pool(name="oh", bufs=4) as opool,
            tc.tile_pool(name="stage", bufs=2) as spool,
            tc.tile_pool(name="dense", bufs=2) as dpool,
            tc.tile_pool(name="mm", bufs=3) as mpool,
            tc.tile_pool(name="pacc", bufs=3, space="PSUM") as ppool,
            tc.tile_pool(name="pt", bufs=1, space="PSUM") as tpool,
            tc.tile_pool(name="po", bufs=2, space="PSUM") as qpool,
        ):
            # ---- constants ----
            wt = {}
            for nm, t_in in w_ins.items():
                t = cpool.tile([D, D], bf16, tag=nm)
                nc.sync.dma_start(t[:], t_in[:])
                wt[nm] = t
            bt = {}
            for nm, t_in in b_ins.items():
                t = cpool.tile([1, D], bf16, tag=nm)
                nc.sync.dma_start(t[:], t_in[:])
                bt[nm] = t
            id64 = cpool.tile([D, D], bf16, tag="id64")
            nc.sync.dma_start(id64[:], id64_in[:])
            identb = cpool.tile([128, 128], bf16, tag="idb")
            nc.sync.dma_start(identb[:], idb_in[:])
            identf = cpool.tile([128, 128], f32, tag="idf")
            nc.sync.dma_start(identf[:], idf_in[:])
            iota_t = cpool.tile([128, 128 * CALL_COLS], bf16, tag="iota")
            nc.sync.dma_start(iota_t[:], iota_in[:])
            ones1 = cpool.tile([1, 128], bf16, tag="ones1")
            nc.gpsimd.memset(ones1[:], 1.0)
            xT_t = cpool.tile([D, NP], bf16, tag="xT")
            for a in range(0, NP, 4096):
                n = min(4096, NP - a)
                nc.sync.dma_start(xT_t[:, a:a + n], xT_in[:, a:a + n])
            gidx = ipool.tile([128, C * 8], i16)
            for a in range(0, C * 8, 2048):
                n = min(2048, C * 8 - a)
                nc.sync.dma_start(gidx[:, a:a + n], gidx_in[:, a:a + n])
            dlow_t = ipool.tile([128, C], bf16)
            nc.sync.dma_start(dlow_t[:], dlow_in[:])

            # ---- gather + one-hot PE scatter-add into window partials ----
            def make_accum(src_dram, parts):
                tiles = {}
                state = {"next": 0}

                def ensure_call(col):
                    while col >= state["next"] * CALL_COLS:
                        g = state["next"]
                        c0 = g * CALL_COLS
                        n = min(CALL_COLS, C - c0)
                        gb = gpool.tile([128, CALL_COLS, D], f32, tag="gb")
                        nc.gpsimd.dma_gather(
                            gb[:, :n, :], src_dram, gidx[:, c0 * 8:(c0 + n) * 8],
                            n * 128, n * 128, D, queue_num=g % 2)
                        gbh = hpool.tile([128, CALL_COLS, D], bf16, tag="gbh")
                        nc.vector.tensor_copy(gbh[:, :n, :], gb[:, :n, :])
                        oh = opool.tile([128, 128, CALL_COLS], bf16, tag="oh")
                        nc.vector.tensor_tensor(
                            oh[:, :, :n],
                            iota_t[:, :].rearrange(
                                "p (d c) -> p d c", c=CALL_COLS)[:, :, :n],
                            dlow_t[:, c0:c0 + n].unsqueeze(1).broadcast_to(
                                (128, 128, n)),
                            Alu.is_equal)
                        for cc in range(c0, c0 + n):
                            tiles[cc] = (oh, gbh, c0)
                        state["next"] += 1

                def accum_pass(k, pending_a2a=None):
                    sg = None
                    for (w, c0, nchw) in sched[WPP * k:WPP * (k + 1)]:
                        if pending_a2a is not None and w % GROUP == GROUP - 1:
                            # dispatch the previous pass's collective right
                            # after this pass's first desc-gens are queued
                            pending_a2a()
                            pending_a2a = None
                        if w % GROUP == 0:
                            pg = ppool.tile([128, GROUP, D], f32, tag="pg")
                        else:
                            pg = state["pg"]
                        state["pg"] = pg
                        ensure_call(c0 + nchw - 1)
                        for i in range(nchw):
                            col = c0 + i
                            oh, gbh, cb = tiles[col]
                            nc.tensor.matmul(
                                pg[:, w % GROUP, :],
                                oh[:, :, col - cb],
                                gbh[:, col - cb, :],
                                start=(i == 0), stop=(i == nchw - 1))
                        if w % GROUP == GROUP - 1:
                            gi = (w % SLAB) // GROUP
                            if gi == 0:
                                sg = spool.tile([128, SLAB, D], bf16, tag="sg")
                            nc.scalar.activation(
                                sg[:, gi * GROUP:(gi + 1) * GROUP, :],
                                pg[:, :, :], AF.Copy)
                            if gi == 1:
                                s = w // SLAB
                                jj = s // NPASS
                                nc.sync.dma_start(
                                    parts[s % NPASS][
                                        1792 * jj:1792 * (jj + 1), :].rearrange(
                                        "(p k) d -> p k d", p=128),
                                    sg[:, :, :])

                return accum_pass

            def a2a_pass(parts, exch, k):
                nc.gpsimd.collective_compute(
                    "AllToAll", mybir.AluOpType.bypass,
                    replica_groups=[list(range(NCORES))],
                    ins=[parts[k][:].opt()],
                    outs=[exch[k][:].opt()])

            # ---- dense: h = lrelu(aggT.T@WrelT + rootT.T@WrootT [+x] + b) ----
            # The 8 senders' partials are summed on the PE: the per-window
            # transpose accumulates all 8 exchanged slabs in PSUM.
            import os
            PASS_MS = float(os.environ.get("PASS_MS", "0.045"))
            vstate = {"layer": 0}

            def dense_pass(exch, wrel, wroot, bias_t, root_dram, residual,
                           out_dram, k):
                if PASS_MS > 0:
                    vw = (vstate["layer"] * (NPASS + 2) + k + DLAG - 0.5) * PASS_MS
                    with tc.tile_wait_until(vw):
                        dense_body(exch, wrel, wroot, bias_t, root_dram,
                                   residual, out_dram, k)
                else:
                    dense_body(exch, wrel, wroot, bias_t, root_dram,
                               residual, out_dram, k)

            def dense_body(exch, wrel, wroot, bias_t, root_dram, residual,
                           out_dram, k):
                ex = dpool.tile([128, NCORES, SLAB, D], bf16, tag="ex")
                nc.scalar.dma_start(
                    ex[:],
                    exch[k][:].rearrange("(j p k) d -> p j k d", p=128, k=SLAB))
                hs = None
                if root_dram is not None:
                    hs = dpool.tile([128, SLAB, D], f32, tag="hs")
                    nc.scalar.dma_start(
                        hs[:],
                        root_dram[1792 * k:1792 * (k + 1), :].rearrange(
                            "(k p) d -> p k d", k=SLAB))
                for kk in range(SLAB):
                    r0 = 1792 * k + 128 * kk
                    pt = tpool.tile([D, 128], f32, tag="pt")
                    for j in range(NCORES):
                        # out[d, n] = sum_e ex[e, d] * I[e, n]  (transpose-acc)
                        nc.tensor.matmul(pt[:], ex[:, j, kk, :], identb[:],
                                         start=(j == 0), stop=(j == NCORES - 1))
                    aT = mpool.tile([D, 128], bf16, tag="aT")
                    nc.scalar.activation(aT[:], pt[:], AF.Copy)
                    if root_dram is not None:
                        pt2 = tpool.tile([D, 128], f32, tag="ptf")
                        nc.tensor.matmul(pt2[:], hs[:, kk, :], identf[:],
                                         start=True, stop=True)
                        rT = mpool.tile([D, 128], bf16, tag="rT")
                        nc.scalar.activation(rT[:], pt2[:], AF.Copy)
                        rTap = rT[:]
                    else:
                        rTap = xT_t[:, r0:r0 + 128]
                    po = qpool.tile([128, D], f32, tag="po")
                    nc.tensor.matmul(po[:], aT[:], wrel[:],
                                     start=True, stop=False)
                    nc.tensor.matmul(po[:], rTap, wroot[:],
                                     start=False, stop=False)
                    if residual:
                        nc.tensor.matmul(po[:], xT_t[:, r0:r0 + 128],
                                         id64[:], start=False, stop=False)
                    nc.tensor.matmul(po[:], ones1[:], bias_t[:],
                                     start=False, stop=True)
                    hrow = mpool.tile([128, D], f32, tag="hrow")
                    nc.scalar.activation(hrow[:], po[:], AF.Lrelu,
                                         alpha=NEG_SLOPE)
                    nc.scalar.dma_start(out_dram[r0:r0 + 128, :], hrow[:])

            # ---- layer driver: lag the collective by 1 pass and dense by 2
            # passes so in-order engine queues never head-of-line block.
            def layer(src_dram, parts, exch, wrel, wroot, bias_t, root_dram,
                      residual, out_dram):
                accum_pass = make_accum(src_dram, parts)
                for k in range(NPASS):
                    pend = None
                    if k >= 1:
                        pend = (lambda kk: lambda: a2a_pass(parts, exch, kk))(k - 1)
                    accum_pass(k, pend)
                    if k >= DLAG:
                        dense_pass(exch, wrel, wroot, bias_t, root_dram,
                                   residual, out_dram, k - DLAG)
                for k in range(NPASS - DLAG, NPASS - 1):
                    dense_pass(exch, wrel, wroot, bias_t, root_dram, residual,
                               out_dram, k)
                a2a_pass(parts, exch, NPASS - 1)
                dense_pass(exch, wrel, wroot, bias_t, root_dram, residual,
                           out_dram, NPASS - 1)

            layer(x_part[:], partials[0], exchs[0], wt["W1relT"], wt["W1rootT"],
                  bt["b1"], None, False, h1)
            vstate["layer"] = 1
            layer(h1[:], partials[1], exchs[1], wt["W2relT"], wt["W2rootT"],
                  bt["b2"], h1, True, y_out)

    nc.compile()
    return nc


def _prep_inputs(x, edge_index, W1_rel, b1, W1_root, W2_rel, b2, W2_root):
    src = np.asarray(edge_index[0]).astype(np.int64)
    dst = np.asarray(edge_index[1]).astype(np.int64)
    plan, gstreams, dstreams, plocal = _make_plan(src, dst)
    plan["plocal"] = plocal
    C = plan["C"]

    x = np.asarray(x, np.float32)
    iota = np.tile(np.repeat(np.arange(128, dtype=np.float32), CALL_COLS),
                   (128, 1)).astype(BF16)
    id64 = np.eye(D, dtype=np.float32).astype(BF16)
    idb = np.eye(128, dtype=np.float32).astype(BF16)
    idf = np.eye(128, dtype=np.float32)

    common = {
        "W1relT": np.ascontiguousarray(np.asarray(W1_rel, np.float32).T).astype(BF16),
        "W1rootT": np.ascontiguousarray(np.asarray(W1_root, np.float32).T).astype(BF16),
        "W2relT": np.ascontiguousarray(np.asarray(W2_rel, np.float32).T).astype(BF16),
        "W2rootT": np.ascontiguousarray(np.asarray(W2_root, np.float32).T).astype(BF16),
        "b1": np.asarray(b1, np.float32).reshape(1, D).astype(BF16),
        "b2": np.asarray(b2, np.float32).reshape(1, D).astype(BF16),
        "id64": id64, "idb": idb, "idf": idf, "iota": iota,
    }

    in_maps = []
    for p in range(NCORES):
        xp = np.zeros((NP, D), np.float32)
        xp[plocal[p]] = x[p * NPART:(p + 1) * NPART]
        m = dict(common)
        m["x_part"] = xp
        m["xT"] = np.ascontiguousarray(xp.T).astype(BF16)
        m["gidx"] = _wrap_stream(gstreams[p])
        m["dlow"] = np.ascontiguousarray(
            dstreams[p].reshape(C, 128).T).astype(BF16)
        in_maps.append(m)
    return plan, in_maps


def kernel(x, edge_index, W1_rel, b1, W1_root, W2_rel, b2, W2_root):
    from concourse import bass_utils

    plan, in_maps = _prep_inputs(x, edge_index, W1_rel, b1, W1_root,
                                 W2_rel, b2, W2_root)
    nc = _build_nc(plan)
    res = bass_utils.run_bass_kernel_spmd(nc, in_maps, core_ids=list(range(NCORES)))
    plocal = plan["plocal"]
    out = np.concatenate(
        [res.results[p]["y"][plocal[p]] for p in range(NCORES)], 0)
    return out.astype(np.float32)
